# revision 1
# baseline (speedup 1.0000x reference)
"""Trainium2 Bass kernel for batched tiny-graph GNN (B=32768, N=22 nodes).

Math per graph (A: [22,22], X: [22,16]):
  H1 = relu(A @ X @ W1 + b1)            -> restructured as relu(((A@X) @ W1) + b1)
  H2 = relu(A @ (H1@W2g) + H1@W2s + b2)
  g  = sum_n H2 ;  y = sigmoid(relu(g@Wf1+bf1) @ Wf2 + bf2)

Mapping: pure data parallelism over 8 cores (4096 graphs each).
On-chip layout: "tetrads" of 4 graphs at 32-aligned partition offsets so
per-graph matmuls get legal auto tile_position; activations kept
channel-major ([C, node-cols]) for weight-stationary f32r dense layers.
"""

import os
import sys

sys.path.insert(0, "/opt/trn_rl_repo")

import numpy as np

import concourse.bass as bass
import concourse.mybir as mybir
import concourse.tile as tile
from concourse.bass_utils import run_bass_kernel_spmd
from concourse.masks import make_identity


import bass_rust


def _patched_drain_and_barrier(self, tick_clock, wait_clock):
    """Walrus in this container cannot encode multi-wait Drain instructions;
    spread the tile-exit sem waits across single-wait NOPs instead."""
    from concourse.tile import ScopedClock

    probe = self.nc.sync.nop(hint="drain_wait_split")
    wait_clock.add_sem_waits(probe.ins, ScopedClock({None: tick_clock.global_clock}))
    si = probe.ins.sync_info
    waits = list(si.on_wait) if si is not None else []
    probe.ins.sync_info = bass_rust.SyncInfo(on_wait=waits[:1], on_update=[])
    for w in waits[1:]:
        n = self.nc.sync.nop(hint="drain_wait_split")
        n.ins.sync_info = bass_rust.SyncInfo(on_wait=[w], on_update=[])
    self.nc.sync.drain()
    self.nc.all_engine_barrier()
    assert self.sems is not None
    popped = self.nc._tile_sem_poison_stack.pop()
    assert popped is self._sem_poison
    self.nc.clear_and_free_semaphores(list(self.sems.allocated().values()))
    self.nc.all_engine_barrier()


tile.TileContext._drain_and_barrier = _patched_drain_and_barrier

F32 = mybir.dt.float32
F32R = mybir.dt.float32r

B_TOTAL = 32768
N_CORES = 8
B_CORE = B_TOTAL // N_CORES          # 4096
NN = 22                              # nodes per graph
FIN = 16
C1, C2, C3 = 128, 64, 32
WAVE = 16                            # graphs per wave (4 tetrads)
AFT = mybir.ActivationFunctionType

# MP2 operand bases differ (lhsT at 32g, rhs at 0); gamble on explicit
# tile_position. If CoreSim disagrees, set False for the safe (slower) path.
MP2_GAMBLE = False



_split_ctr = [0]


def _split_multi_waits(nc):
    """This container's walrus encodes at most one sem wait per instruction:
    hoist extra waits onto same-engine NOPs inserted just before."""
    for f in nc.m.functions:
        for bb in f.blocks:
            out = []
            for inst in bb.instructions:
                si = inst.sync_info
                if si is not None and len(si.on_wait) > 1:
                    waits = list(si.on_wait)
                    for w in waits[:-1]:
                        _split_ctr[0] += 1
                        n = mybir.InstNoOp(
                            name=f"waitsplit_{_split_ctr[0]}", ins=[], outs=[]
                        )
                        n.engine = inst.engine
                        n.sync_info = bass_rust.SyncInfo(on_wait=[w], on_update=[])
                        out.append(n)
                        nc.register_instruction(n, overwrite=True)
                    inst.sync_info = bass_rust.SyncInfo(
                        on_wait=[waits[-1]], on_update=list(si.on_update)
                    )
                out.append(inst)
            bb.instructions = out


def build_nc(b_core: int = B_CORE) -> bass.Bass:
    assert b_core % WAVE == 0
    n_waves = b_core // WAVE

    nc = bass.Bass()

    a_d = nc.declare_dram_parameter("a", [b_core, NN, NN], F32, isOutput=False)
    x_d = nc.declare_dram_parameter("x", [b_core, NN, FIN], F32, isOutput=False)
    w1_d = nc.declare_dram_parameter("w1", [FIN, C1], F32, isOutput=False)
    b1_d = nc.declare_dram_parameter("b1", [C1], F32, isOutput=False)
    w2g_d = nc.declare_dram_parameter("w2g", [C1, C2], F32, isOutput=False)
    w2s_d = nc.declare_dram_parameter("w2s", [C1, C2], F32, isOutput=False)
    b2_d = nc.declare_dram_parameter("b2", [C2], F32, isOutput=False)
    wf1_d = nc.declare_dram_parameter("wf1", [C2, C3], F32, isOutput=False)
    bf1_d = nc.declare_dram_parameter("bf1", [C3], F32, isOutput=False)
    wf2_d = nc.declare_dram_parameter("wf2", [C3, 1], F32, isOutput=False)
    bf2_d = nc.declare_dram_parameter("bf2", [1], F32, isOutput=False)
    y_d = nc.declare_dram_parameter("y", [b_core, 1], F32, isOutput=True)

    with tile.TileContext(nc) as tc:
        with (
            tc.tile_pool(name="const", bufs=1) as cpool,
            tc.tile_pool(name="ioa", bufs=3) as iopool,
            tc.tile_pool(name="iox", bufs=3) as ioxpool,
            tc.tile_pool(name="work", bufs=2) as wpool,
            tc.tile_pool(name="ps", bufs=1, space="PSUM") as pspool,
            tc.tile_pool(name="ps2", bufs=2, space="PSUM") as ps2pool,
        ):
            # ---- persistent constants ----
            ident = cpool.tile([128, 128], F32, tag="ident")
            make_identity(nc, ident[:, :])
            w1sb_t = cpool.tile([128, C1], F32, tag="w1")
            w1sb = w1sb_t[0:FIN]
            nc.sync.dma_start(out=w1sb[:, :], in_=w1_d[:, :])
            w2sb = cpool.tile([C1, 128], F32, tag="w2")
            nc.sync.dma_start(out=w2sb[:, 0:C2], in_=w2g_d[:, :])
            nc.sync.dma_start(out=w2sb[:, C2:128], in_=w2s_d[:, :])
            wf1sb_t = cpool.tile([128, C3], F32, tag="wf1")
            wf1sb = wf1sb_t[0:C2]
            nc.sync.dma_start(out=wf1sb[:, :], in_=wf1_d[:, :])
            wf2sb_t = cpool.tile([128, 1], F32, tag="wf2")
            wf2sb = wf2sb_t[0:C3]
            nc.sync.dma_start(out=wf2sb[:, :], in_=wf2_d[:, :])
            b1c = cpool.tile([C1, 1], F32, tag="b1")
            nc.sync.dma_start(out=b1c[:, :], in_=b1_d.rearrange("(c o) -> c o", o=1))
            # bias for [Zg | Zs] rows: 0 for Zg rows, b2 for Zs rows
            b2e = cpool.tile([128, 1], F32, tag="b2")
            nc.vector.memset(b2e[:, :], 0.0)
            nc.sync.dma_start(out=b2e[C2:128, :], in_=b2_d.rearrange("(c o) -> c o", o=1))
            bf1c_t = cpool.tile([128, 1], F32, tag="bf1")
            bf1c = bf1c_t[0:C3]
            nc.sync.dma_start(out=bf1c[:, :], in_=bf1_d.rearrange("(c o) -> c o", o=1))
            bf2c_t = cpool.tile([128, 1], F32, tag="bf2")
            bf2c = bf2c_t[0:1]
            nc.sync.dma_start(out=bf2c[:, :], in_=bf2_d.rearrange("(c o) -> c o", o=1))
            y_acc = cpool.tile([1, b_core], F32, tag="yacc")

            for w in range(n_waves):
                b0 = w * WAVE
                # ---- input DMAs ----
                # A: tetrad-stacked natural [128(32g+n), (t,m)]
                a_wave = iopool.tile([128, 4 * NN], F32, tag="a_wave")
                a_quad = a_d[b0 : b0 + WAVE].rearrange("(t g) n m -> g n t m", t=4)
                for g in range(4):
                    nc.sync.dma_start(
                        out=a_wave[32 * g : 32 * g + NN].rearrange(
                            "p (t m) -> p t m", t=4
                        ),
                        in_=a_quad[g],
                    )
                # X: horizontal [22(m), (j graphs, f)]
                x_wave_t = ioxpool.tile([128, WAVE * FIN], F32, tag="x_wave")
                x_wave = x_wave_t[0:NN]
                x_dst = x_wave.rearrange("p (j f) -> p j f", j=WAVE)
                x_src = x_d[b0 : b0 + WAVE].rearrange("j m f -> m j f")
                nc.sync.dma_start(out=x_dst, in_=x_src)

                # ---- A transposes: sAT[22(m), (t, 32g+n)] ----
                sAT_t = wpool.tile([128, 4 * 4 * NN], F32, tag="sAT")
                sAT = sAT_t[0:NN]
                for t in range(4):
                    pAT = pspool.tile([NN, 128], F32, tag="pat")
                    nc.tensor.transpose(
                        pAT[:, :],
                        a_wave[:, t * NN : (t + 1) * NN],
                        ident[:, :],
                    )
                    nc.scalar.copy(
                        out=sAT[:, t * 88 : (t + 1) * 88].rearrange(
                            "p (g n) -> p g n", g=4
                        ),
                        in_=pAT.rearrange("p (g q) -> p g q", g=4)[:, :, 0:NN],
                    )

                # ---- MP1 + transpose to channel-major AXT [16, 512] ----
                sAXT_t = wpool.tile([128, 512], F32, tag="sAXT")
                sAXT = sAXT_t[0:FIN]
                for t in range(4):
                    pAX = ps2pool.tile([128, FIN], F32, tag="pax")
                    nc.vector.memset(pAX[:, :], 0.0)
                    for g in range(4):
                        j = 4 * t + g
                        nc.tensor.matmul(
                            pAX[32 * g : 32 * g + NN, :],
                            lhsT=sAT[:, t * 88 + 22 * g : t * 88 + 22 * g + NN],
                            rhs=x_wave[:, j * FIN : (j + 1) * FIN],
                            tile_position=(0, 32 * g),
                        )
                    sAX = wpool.tile([128, FIN], F32, tag="sAX")
                    nc.vector.tensor_copy(sAX[:, :], pAX[:, :])
                    pAXT = pspool.tile([FIN, 128], F32, tag="paxt")
                    nc.tensor.transpose(
                        pAXT[:, :], sAX[:, :],
                        ident[:, :],
                    )
                    nc.scalar.copy(out=sAXT[:, t * 128 : (t + 1) * 128], in_=pAXT[:, :])

                # ---- dense1 (f32r): Z1T = (AX @ W1)^T ; relu+bias -> H1T ----
                pZ1T = pspool.tile([C1, 512], F32, tag="z1t")
                nc.tensor.matmul(
                    pZ1T[:, :], lhsT=w1sb[:, :], rhs=sAXT[:, :]
                )
                sH1T = wpool.tile([C1, 512], F32, tag="sH1T")
                nc.scalar.activation(
                    out=sH1T[:, :], in_=pZ1T[:, :], func=AFT.Relu, bias=b1c[:, :]
                )

                # ---- dense2 (f32r): [ZgT; ZsT] = (H1 @ [W2g|W2s])^T (+bias on Zs) ----
                pZT = pspool.tile([128, 512], F32, tag="zt")
                nc.tensor.matmul(
                    pZT[:, :], lhsT=w2sb[:, :], rhs=sH1T[:, :]
                )
                sZ = wpool.tile([128, 512], F32, tag="sZ")
                nc.vector.tensor_scalar(
                    out=sZ[:, :], in0=pZT[:, :], scalar1=b2e[:, :], scalar2=None,
                    op0=mybir.AluOpType.add,
                )

                # ---- MP2: pAZgT[64, (t, 32g+n)] = (A @ Zg)^T per graph ----
                pAZgT = pspool.tile([C2, 512], F32, tag="azgt")
                for t in range(4):
                    for g in range(4):
                        if MP2_GAMBLE:
                            nc.tensor.matmul(
                                pAZgT[:, t * 128 + 32 * g : t * 128 + 32 * g + NN],
                                lhsT=sZg[32 * g : 32 * g + NN, t * C2 : (t + 1) * C2],
                                rhs=sAT[:, t * 88 + 22 * g : t * 88 + 22 * g + NN],
                                tile_position=(0, 0),
                                skip_group_check=True,
                            )
                        else:
                            # safe path: re-transpose Zg slice to base 0 first
                            pZn = pspool.tile([NN, C2], F32, tag="pat")
                            nc.tensor.transpose(
                                pZn[:, :],
                                sZ[0:C2, t * 128 + 32 * g : t * 128 + 32 * g + NN],
                                ident[0:C2, 0:C2],
                            )
                            sZn = wpool.tile([NN, C2], F32, tag="sZn")
                            nc.vector.tensor_copy(sZn[:, :], pZn[:, :])
                            nc.tensor.matmul(
                                pAZgT[:, t * 128 + 32 * g : t * 128 + 32 * g + NN],
                                lhsT=sZn[:, :],
                                rhs=sAT[:, t * 88 + 22 * g : t * 88 + 22 * g + NN],
                            )

                # ---- H2T = relu(AZgT + ZsT(+b2)) on valid cols, compacted ----
                azg_v = pAZgT.rearrange("p (t q n) -> p t q n", t=4, q=4)[:, :, :, 0:NN]
                zs_v = sZ[C2:128, :].rearrange("p (t q n) -> p t q n", t=4, q=4)[:, :, :, 0:NN]
                sH2T_t = wpool.tile([128, WAVE * NN], F32, tag="sH2T")
                sH2T = sH2T_t[0:C2]
                h2_v = sH2T.rearrange("p (t q n) -> p t q n", t=4, q=4)
                nc.vector.tensor_tensor(
                    out=h2_v, in0=azg_v, in1=zs_v, op=mybir.AluOpType.add
                )
                nc.scalar.activation(
                    out=sH2T[:, :], in_=sH2T[:, :], func=AFT.Relu
                )

                # ---- pool over nodes + final MLP ----
                sG_t = wpool.tile([128, WAVE], F32, tag="sG")
                sG = sG_t[0:C2]
                nc.vector.reduce_sum(
                    out=sG[:, :],
                    in_=sH2T.rearrange("p (j n) -> p j n", j=WAVE),
                    axis=mybir.AxisListType.X,
                )
                pG1 = pspool.tile([C3, WAVE], F32, tag="pat")
                nc.tensor.matmul(pG1[:, :], lhsT=wf1sb[:, :], rhs=sG[:, :])
                sG1_t = wpool.tile([128, WAVE], F32, tag="sG1")
                sG1 = sG1_t[0:C3]
                nc.scalar.activation(
                    out=sG1[:, :], in_=pG1[:, :], func=AFT.Relu, bias=bf1c[:, :]
                )
                pY = pspool.tile([1, WAVE], F32, tag="paxt")
                nc.tensor.matmul(pY[:, :], lhsT=wf2sb[:, :], rhs=sG1[:, :])
                nc.scalar.activation(
                    out=y_acc[:, b0 : b0 + WAVE], in_=pY[:, :], func=AFT.Sigmoid,
                    bias=bf2c[:, :],
                )

            nc.sync.dma_start(
                out=y_d.rearrange("(o b) one -> o (b one)", o=1), in_=y_acc[:, :]
            )

    _split_multi_waits(nc)
    return nc


def kernel(**inputs) -> np.ndarray:
    x = np.asarray(inputs["x"], dtype=np.float32)
    a = np.asarray(inputs["a"], dtype=np.float32)
    weights = {
        k: np.asarray(inputs[k], dtype=np.float32)
        for k in ("w1", "b1", "w2g", "w2s", "b2", "wf1", "bf1", "wf2", "bf2")
    }

    nc = build_nc(B_CORE)
    in_maps = []
    for c in range(N_CORES):
        sl = slice(c * B_CORE, (c + 1) * B_CORE)
        m = {"x": x[sl], "a": a[sl]}
        m.update(weights)
        in_maps.append(m)

    res = run_bass_kernel_spmd(nc, in_maps, list(range(N_CORES)))
    outs = [res.results[c]["y"] for c in range(N_CORES)]
    return np.concatenate(outs, axis=0).astype(np.float32)


if __name__ == "__main__":
    rng = np.random.default_rng(0)
    demo = {
        "x": rng.standard_normal((B_TOTAL, NN, FIN), dtype=np.float32),
        "a": rng.random((B_TOTAL, NN, NN), dtype=np.float32),
        "w1": rng.standard_normal((FIN, C1), dtype=np.float32) * 0.1,
        "b1": np.zeros(C1, np.float32),
        "w2g": rng.standard_normal((C1, C2), dtype=np.float32) * 0.1,
        "w2s": rng.standard_normal((C1, C2), dtype=np.float32) * 0.1,
        "b2": np.zeros(C2, np.float32),
        "wf1": rng.standard_normal((C2, C3), dtype=np.float32) * 0.1,
        "bf1": np.zeros(C3, np.float32),
        "wf2": rng.standard_normal((C3, 1), dtype=np.float32) * 0.1,
        "bf2": np.zeros(1, np.float32),
    }
    y = kernel(**demo)
    print("out", y.shape, y.dtype, y[:4, 0])



# revision 20
# speedup vs baseline: 4.5806x; 4.5806x over previous
"""Trainium2 Bass kernel for batched tiny-graph GNN (B=32768, N=22 nodes).

Math per graph (A: [22,22], X: [22,16]):
  H1 = relu((A @ X) @ W1 + b1)                [22,128]
  H2 = relu(A @ (H1@W2g) + H1@W2s + b2)       [22,64]
  y  = sigmoid(relu(sum_n H2 @ Wf1 + bf1) @ Wf2 + bf2)

Design (per core, 4096 graphs padded to 4160 = 52 superwaves x 80):
  - MP1 (A@X): blockdiag-X lhsT [88,128] (4 graphs at 22-part/32-col
    offsets) x A^T stacked rhs -> AXT channel-major, fp16 with hi/lo
    residual passes (3 matmuls) for near-f32 accuracy.
  - dense1/dense2: channel-major f32r (11-bit mantissa, 1 cycle/row at
    free dim 440 >= 256). Weights host-rounded to 11 bits.
  - Z -> node-major via PE transpose (fp16), MP2 = blockdiag-A^T lhsT
    [111,110] fp16 (extra ones-row folds b2 in) x Zg-node rhs.
  - Pooling via mask matmul on PE; final MLP in plain f32.
Host side: pad, transpose A, split fp16 hi/lo, build block layouts;
outputs inverse-permuted and sliced back to 4096 per core.
"""

import sys

sys.path.insert(0, "/opt/trn_rl_repo")

import numpy as np

import concourse.bass as bass
import concourse.mybir as mybir
import concourse.tile as tile
from concourse.bass_utils import run_bass_kernel_spmd

import bass_rust


def _patched_drain_and_barrier(self, tick_clock, wait_clock):
    """Walrus in this container cannot encode multi-wait drain instructions;
    spread the tile-exit sem waits across single-wait NOPs instead."""
    from concourse.tile import ScopedClock

    probe = self.nc.sync.nop(hint="drain_wait_split")
    wait_clock.add_sem_waits(probe.ins, ScopedClock({None: tick_clock.global_clock}))
    si = probe.ins.sync_info
    waits = list(si.on_wait) if si is not None else []
    probe.ins.sync_info = bass_rust.SyncInfo(on_wait=waits[:1], on_update=[])
    for w in waits[1:]:
        n = self.nc.sync.nop(hint="drain_wait_split")
        n.ins.sync_info = bass_rust.SyncInfo(on_wait=[w], on_update=[])
    self.nc.sync.drain()
    self.nc.all_engine_barrier()
    assert self.sems is not None
    popped = self.nc._tile_sem_poison_stack.pop()
    assert popped is self._sem_poison
    self.nc.clear_and_free_semaphores(list(self.sems.allocated().values()))
    self.nc.all_engine_barrier()


tile.TileContext._drain_and_barrier = _patched_drain_and_barrier

_split_ctr = [0]


def _split_multi_waits(nc):
    """This container's walrus encodes at most one sem wait per instruction:
    hoist extra waits onto same-engine NOPs inserted just before."""
    for f in nc.m.functions:
        for bb in f.blocks:
            out = []
            for inst in bb.instructions:
                si = inst.sync_info
                if si is not None and len(si.on_wait) > 1:
                    waits = list(si.on_wait)
                    for w in waits[:-1]:
                        _split_ctr[0] += 1
                        n = mybir.InstNoOp(
                            name=f"waitsplit_{_split_ctr[0]}", ins=[], outs=[]
                        )
                        n.engine = inst.engine
                        n.sync_info = bass_rust.SyncInfo(on_wait=[w], on_update=[])
                        out.append(n)
                        nc.register_instruction(n, overwrite=True)
                    inst.sync_info = bass_rust.SyncInfo(
                        on_wait=[waits[-1]], on_update=list(si.on_update)
                    )
                out.append(inst)
            bb.instructions = out


F32 = mybir.dt.float32
F32R = mybir.dt.float32r
F16 = mybir.dt.float16
AFT = mybir.ActivationFunctionType

B_TOTAL = 32768
N_CORES = 8
B_CORE = B_TOTAL // N_CORES          # 4096
NN = 22
FIN = 16
C1, C2, C3 = 128, 64, 32

SW_G = 80                            # graphs per superwave
B_PAD = 4160                         # 52 superwaves x 80
N_SW = B_PAD // SW_G                 # 52
NQ = 20                              # 4-graph groups per superwave
NSET = 4                             # 5-graph sets per slot
FREE = NQ * NN                       # 440 dense free dim


def round11(v):
    """Round f32 to 11 mantissa bits (what the PE does for f32r operands)."""
    v = np.ascontiguousarray(v, np.float32)
    b = v.view(np.uint32).astype(np.uint64)
    b = ((b + np.uint64(1 << 11)) >> np.uint64(12)) << np.uint64(12)
    return b.astype(np.uint32).view(np.float32)


def build_nc() -> bass.Bass:
    nc = bass.Bass()

    x32_d = nc.declare_dram_parameter("x32", [B_PAD, NN, FIN], F32, isOutput=False)
    at32_d = nc.declare_dram_parameter("at32", [B_PAD, NN, NN], F32, isOutput=False)
    athi_d = nc.declare_dram_parameter("athi", [B_PAD, NN, NN], F16, isOutput=False)
    atlo_d = nc.declare_dram_parameter("atlo", [B_PAD, NN, NN], F16, isOutput=False)
    w1rep_d = nc.declare_dram_parameter("w1rep", [128, 128], F32R, isOutput=False)
    w1lor_d = nc.declare_dram_parameter("w1lor", [128, 128], F32R, isOutput=False)
    w2sb_d = nc.declare_dram_parameter("w2sb", [128, 128], F32R, isOutput=False)
    w2lo_d = nc.declare_dram_parameter("w2lo", [128, 128], F32R, isOutput=False)
    ident_d = nc.declare_dram_parameter("ident", [128, 128], F16, isOutput=False)
    mask5_d = nc.declare_dram_parameter("mask5", [110, 5], F32, isOutput=False)
    b1_d = nc.declare_dram_parameter("b1c", [C1, 1], F32, isOutput=False)
    b2blk_d = nc.declare_dram_parameter("b2blk", [1, 16 * 128], F16, isOutput=False)
    ones_d = nc.declare_dram_parameter("ones1", [1, 16 * 110], F16, isOutput=False)
    bf1_d = nc.declare_dram_parameter("bf1c", [C3, 1], F32, isOutput=False)
    bf2_d = nc.declare_dram_parameter("bf2c", [1, 1], F32, isOutput=False)
    wf1_d = nc.declare_dram_parameter("wf1", [C2, C3], F32, isOutput=False)
    wf2_d = nc.declare_dram_parameter("wf2", [C3, 1], F32, isOutput=False)
    y_d = nc.declare_dram_parameter("y", [B_PAD, 1], F32, isOutput=True)

    # dram views for the superwave DMAs (slot-major: graph = B0 + 20*gi + q)
    x32_v = x32_d.rearrange("(b gi q) m f -> b gi m q f", gi=4, q=NQ)
    at32_v = at32_d.rearrange("(b gi q) m n -> b gi m q n", gi=4, q=NQ)
    # blockdiag-A sets: set k = 4*gi+s holds graphs {B0 + 5k + j : j=0..4}
    atbd_v = athi_d.rearrange("(b k j) m n -> b j m k n", k=16, j=5)
    atbdlo_v = atlo_d.rearrange("(b k j) m n -> b j m k n", k=16, j=5)

    with tile.TileContext(nc) as tc:
        with (
            tc.tile_pool(name="const", bufs=1) as cpool,
            tc.tile_pool(name="io", bufs=2) as iopool,
            tc.tile_pool(name="work", bufs=2) as wpool,
        ):
            # ---- persistent constants ----
            w1rep = cpool.tile([128, 128], F32R, tag="w1rep")
            nc.sync.dma_start(out=w1rep[:, :], in_=w1rep_d[:, :])
            w1lor = cpool.tile([128, 128], F32R, tag="w1lor")
            nc.sync.dma_start(out=w1lor[:, :], in_=w1lor_d[:, :])
            w2sb = cpool.tile([128, 128], F32R, tag="w2sb")
            nc.sync.dma_start(out=w2sb[:, :], in_=w2sb_d[:, :])
            w2lo = cpool.tile([128, 128], F32R, tag="w2lo")
            nc.sync.dma_start(out=w2lo[:, :], in_=w2lo_d[:, :])
            ident = cpool.tile([128, 128], F16, tag="ident")
            nc.sync.dma_start(out=ident[:, :], in_=ident_d[:, :])
            mask5 = cpool.tile([110, 5], F32, tag="mask5")
            nc.sync.dma_start(out=mask5[:, :], in_=mask5_d[:, :])
            b1c = cpool.tile([C1, 1], F32, tag="b1c")
            nc.sync.dma_start(out=b1c[:, :], in_=b1_d[:, :])
            bf1c = cpool.tile([C3, 1], F32, tag="bf1c")
            nc.sync.dma_start(out=bf1c[:, :], in_=bf1_d[:, :])
            bf2c = cpool.tile([1, 1], F32, tag="bf2c")
            nc.sync.dma_start(out=bf2c[:, :], in_=bf2_d[:, :])
            wf1sb = cpool.tile([C2, C3], F32, tag="wf1")
            nc.sync.dma_start(out=wf1sb[:, :], in_=wf1_d[:, :])
            wf2sb = cpool.tile([C3, 1], F32, tag="wf2")
            nc.sync.dma_start(out=wf2sb[:, :], in_=wf2_d[:, :])
            sG = cpool.tile([C2, B_PAD], F32, tag="sG")
            sY = cpool.tile([1, B_PAD], F32, tag="sY")

            # ---- prologue: zero blockdiag buffers, seed static rows ----
            for _ in range(2):
                bdX = iopool.tile([88, NQ * 128], F32, tag="bdX")
                nc.vector.memset(bdX[:, :], 0.0)
                bdA = iopool.tile([111, 16 * 110], F16, tag="bdA")
                nc.vector.memset(bdA[0:110, :], 0.0)
                nc.sync.dma_start(out=bdA[110:111, :], in_=ones_d[:, :])
                bdAlo = iopool.tile([111, 16 * 110], F16, tag="bdAlo")
                nc.vector.memset(bdAlo[:, :], 0.0)
                sZn = wpool.tile([111, 16 * 128], F16, tag="sZn")
                nc.sync.dma_start(out=sZn[110:111, :], in_=b2blk_d[:, :])

            # ---- main loop over superwaves ----
            main_psum = tc.tile_pool(name="big", bufs=4, space="PSUM")
            bigpool = main_psum.__enter__()
            pzn_ctx = tc.tile_pool(name="pzn", bufs=2, space="PSUM")
            pznpool = pzn_ctx.__enter__()
            paz_ctx = tc.tile_pool(name="pazg", bufs=1, space="PSUM")
            pazpool = paz_ctx.__enter__()
            pg_ctx = tc.tile_pool(name="pg", bufs=1, space="PSUM")
            pgpool = pg_ctx.__enter__()
            for sw in range(N_SW):
                bdX = iopool.tile([88, NQ * 128], F32, tag="bdX")
                ATv = iopool.tile([88, FREE], F32, tag="ATv")
                bdA = iopool.tile([111, 16 * 110], F16, tag="bdA")
                bdAlo = iopool.tile([111, 16 * 110], F16, tag="bdAlo")

                for gi in range(4):
                    dstx = bdX[22 * gi : 22 * gi + NN].rearrange(
                        "p (q blk) -> p q blk", q=NQ
                    )[:, :, 32 * gi : 32 * gi + FIN]
                    nc.sync.dma_start(out=dstx, in_=x32_v[sw, gi])
                    nc.sync.dma_start(
                        out=ATv[22 * gi : 22 * gi + NN].rearrange(
                            "p (q n) -> p q n", q=NQ
                        ),
                        in_=at32_v[sw, gi],
                    )
                for j in range(5):
                    dst = bdA[22 * j : 22 * j + NN].rearrange(
                        "p (k nn) -> p k nn", k=16
                    )[:, :, 22 * j : 22 * j + NN]
                    nc.sync.dma_start(out=dst, in_=atbd_v[sw, j])
                    dstlo = bdAlo[22 * j : 22 * j + NN].rearrange(
                        "p (k nn) -> p k nn", k=16
                    )[:, :, 22 * j : 22 * j + NN]
                    nc.sync.dma_start(out=dstlo, in_=atbdlo_v[sw, j])

                # ---- MP1: AXT channel-major [128, 440], fp16 x3 passes ----
                pAXT = bigpool.tile([128, FREE], F32, tag="big")
                for q in range(NQ):
                    nc.tensor.matmul(
                        pAXT[:, NN * q : NN * q + NN],
                        lhsT=bdX[:, 128 * q : 128 * q + 128],
                        rhs=ATv[:, NN * q : NN * q + NN],
                    )

                sAXT = wpool.tile([128, FREE], F32R, tag="sAXT")
                nc.vector.tensor_copy(sAXT[:, :], pAXT[:, :])

                sH1T = wpool.tile([128, 4 * FREE], F32R, tag="sH1T")
                sZT = wpool.tile([128, 4 * FREE], F16, tag="sZT")
                sZn = wpool.tile([111, 16 * 128], F16, tag="sZn")
                sH2 = wpool.tile([110, 16 * C2], F32, tag="sH2")
                h2t = wpool.tile([110, 16 * C2], F32, tag="h2t")
                pG = pgpool.tile([C2, SW_G], F32, tag="pg")

                for gi in range(4):
                    # dense1 (f32r, free=440)
                    pH1T = bigpool.tile([128, FREE], F32, tag="big")
                    nc.tensor.matmul(
                        pH1T[:, :],
                        lhsT=w1rep[32 * gi : 32 * gi + FIN, :],
                        rhs=sAXT[32 * gi : 32 * gi + FIN, :],
                        tile_position=(32 * gi, 0),
                        start=True, stop=False,
                    )
                    nc.tensor.matmul(
                        pH1T[:, :],
                        lhsT=w1lor[32 * gi : 32 * gi + FIN, :],
                        rhs=sAXT[32 * gi : 32 * gi + FIN, :],
                        tile_position=(32 * gi, 0),
                        start=False, stop=True,
                    )
                    h1sl = sH1T[:, FREE * gi : FREE * gi + FREE]
                    if gi % 2 == 0:
                        nc.scalar.activation(
                            out=h1sl, in_=pH1T[:, :], func=AFT.Relu, bias=b1c[:, :]
                        )
                    else:
                        nc.vector.tensor_scalar(
                            out=h1sl, in0=pH1T[:, :], scalar1=b1c[:, :], scalar2=0.0,
                            op0=mybir.AluOpType.add, op1=mybir.AluOpType.max,
                        )

                    # dense2 (f32r, free=440) -> [Zg|Zs] channel-major
                    pZT = bigpool.tile([128, FREE], F32, tag="big")
                    nc.tensor.matmul(
                        pZT[:, :], lhsT=w2sb[:, :], rhs=h1sl, start=True, stop=False
                    )
                    nc.tensor.matmul(
                        pZT[:, :], lhsT=w2lo[:, :], rhs=h1sl, start=False, stop=True
                    )
                    ztsl = sZT[:, FREE * gi : FREE * gi + FREE]
                    nc.scalar.copy(out=ztsl, in_=pZT[:, :])

                    # per 5-graph set: transpose -> MP2 -> H2 -> pool
                    pZn = pznpool.tile([110, 4 * 128], F16, tag="pzn")
                    pAZ = pazpool.tile([110, 4 * C2], F32, tag="pazg")
                    for s in range(NSET):
                        nc.tensor.transpose(
                            pZn[:, 128 * s : 128 * s + 128],
                            sZT[:, FREE * gi + 110 * s : FREE * gi + 110 * s + 110],
                            ident[:, :],
                        )
                        k = 4 * gi + s
                        nc.scalar.copy(
                            out=sZn[0:110, 128 * k : 128 * k + 128],
                            in_=pZn[:, 128 * s : 128 * s + 128],
                        )
                        nc.tensor.matmul(
                            pAZ[:, C2 * s : C2 * s + C2],
                            lhsT=bdA[:, 110 * k : 110 * k + 110],
                            rhs=sZn[:, 128 * k : 128 * k + C2],
                            start=True, stop=False,
                        )
                        nc.tensor.matmul(
                            pAZ[:, C2 * s : C2 * s + C2],
                            lhsT=bdAlo[:, 110 * k : 110 * k + 110],
                            rhs=sZn[:, 128 * k : 128 * k + C2],
                            start=False, stop=True,
                        )
                        h2sl = h2t[:, C2 * k : C2 * k + C2]
                        nc.vector.tensor_tensor(
                            out=h2sl,
                            in0=pAZ[:, C2 * s : C2 * s + C2],
                            in1=sZn[0:110, 128 * k + C2 : 128 * k + 128],
                            op=mybir.AluOpType.add,
                        )
                        hh = sH2[:, C2 * k : C2 * k + C2]
                        nc.gpsimd.tensor_scalar(
                            out=hh, in0=h2sl, scalar1=0.0, scalar2=None,
                            op0=mybir.AluOpType.max,
                        )
                        nc.tensor.matmul(
                            pG[:, 5 * k : 5 * k + 5],
                            lhsT=hh,
                            rhs=mask5[:, :],
                        )

                nc.vector.tensor_copy(sG[:, SW_G * sw : SW_G * sw + SW_G], pG[:, :])

            pg_ctx.__exit__(None, None, None)
            paz_ctx.__exit__(None, None, None)
            pzn_ctx.__exit__(None, None, None)
            main_psum.__exit__(None, None, None)

            # ---- final MLP over all graphs (f32) ----
            with tc.tile_pool(name="fin", bufs=2, space="PSUM") as fpool:
                CH = 416
                for c in range(B_PAD // CH):
                    sl = slice(CH * c, CH * c + CH)
                    pF = fpool.tile([C3, CH], F32, tag="pf")
                    nc.tensor.matmul(pF[:, :], lhsT=wf1sb[:, :], rhs=sG[:, sl])
                    sF = wpool.tile([C3, CH], F32, tag="sF")
                    nc.scalar.activation(
                        out=sF[:, :], in_=pF[:, :], func=AFT.Relu, bias=bf1c[:, :]
                    )
                    pY = fpool.tile([1, CH], F32, tag="py")
                    nc.tensor.matmul(pY[:, :], lhsT=wf2sb[:, :], rhs=sF[:, :])
                    nc.scalar.activation(
                        out=sY[:, sl], in_=pY[:, :], func=AFT.Sigmoid, bias=bf2c[:, :]
                    )

            nc.sync.dma_start(
                out=y_d.rearrange("(o b) one -> o (b one)", o=1), in_=sY[:, :]
            )

    _split_multi_waits(nc)
    return nc


def _f16(v):
    return np.ascontiguousarray(v, np.float32).astype(np.float16)


def prep_in_maps(inputs):
    x = np.ascontiguousarray(inputs["x"], np.float32)
    a = np.ascontiguousarray(inputs["a"], np.float32)
    w1 = np.asarray(inputs["w1"], np.float32)
    b1 = np.asarray(inputs["b1"], np.float32)
    w2g = np.asarray(inputs["w2g"], np.float32)
    w2s = np.asarray(inputs["w2s"], np.float32)
    b2 = np.asarray(inputs["b2"], np.float32)
    wf1 = np.asarray(inputs["wf1"], np.float32)
    bf1 = np.asarray(inputs["bf1"], np.float32)
    wf2 = np.asarray(inputs["wf2"], np.float32)
    bf2 = np.asarray(inputs["bf2"], np.float32)

    w1rep = np.zeros((128, 128), np.float32)
    w1lor = np.zeros((128, 128), np.float32)
    for gi in range(4):
        w1rep[32 * gi : 32 * gi + FIN, :] = round11(w1)
        w1lor[32 * gi : 32 * gi + FIN, :] = round11(w1 - round11(w1))
    w2cat = np.concatenate([w2g, w2s], axis=1)
    w2sb = round11(w2cat)
    w2lo = round11(w2cat - w2sb)
    ident = np.eye(128, dtype=np.float16)
    mask5 = np.zeros((110, 5), np.float32)
    for j in range(5):
        mask5[22 * j : 22 * j + NN, j] = 1.0
    b2blk = np.zeros((16, 128), np.float16)
    b2blk[:, :C2] = _f16(b2)[None, :]
    b2blk = b2blk.reshape(1, 16 * 128)

    const = {
        "w1rep": w1rep,
        "w1lor": w1lor,
        "w2sb": w2sb,
        "w2lo": w2lo,
        "ident": ident,
        "mask5": mask5,
        "b1c": b1.reshape(C1, 1),
        "b2blk": b2blk,
        "ones1": np.ones((1, 16 * 110), np.float16),
        "bf1c": bf1.reshape(C3, 1),
        "bf2c": bf2.reshape(1, 1),
        "wf1": wf1,
        "wf2": wf2.reshape(C3, 1),
    }

    in_maps = []
    for c in range(N_CORES):
        sl = slice(c * B_CORE, (c + 1) * B_CORE)
        xc = np.zeros((B_PAD, NN, FIN), np.float32)
        xc[:B_CORE] = x[sl]
        ac = np.zeros((B_PAD, NN, NN), np.float32)
        ac[:B_CORE] = a[sl]
        at = np.ascontiguousarray(ac.transpose(0, 2, 1))
        athi = _f16(at)
        atlo = _f16(at - athi.astype(np.float32))
        m = {"x32": xc, "at32": at, "athi": athi, "atlo": atlo}
        m.update(const)
        in_maps.append(m)
    return in_maps


def kernel(**inputs) -> np.ndarray:
    in_maps = prep_in_maps(inputs)
    nc = build_nc()
    res = run_bass_kernel_spmd(nc, in_maps, list(range(N_CORES)))
    outs = [np.asarray(res.results[c]["y"])[:B_CORE] for c in range(N_CORES)]
    return np.concatenate(outs, axis=0).astype(np.float32)


if __name__ == "__main__":
    rng = np.random.default_rng(0)
    demo = {
        "x": rng.standard_normal((B_TOTAL, NN, FIN), dtype=np.float32),
        "a": rng.random((B_TOTAL, NN, NN), dtype=np.float32),
        "w1": rng.standard_normal((FIN, C1), dtype=np.float32) * 0.1,
        "b1": np.zeros(C1, np.float32),
        "w2g": rng.standard_normal((C1, C2), dtype=np.float32) * 0.1,
        "w2s": rng.standard_normal((C1, C2), dtype=np.float32) * 0.1,
        "b2": np.zeros(C2, np.float32),
        "wf1": rng.standard_normal((C2, C3), dtype=np.float32) * 0.1,
        "bf1": np.zeros(C3, np.float32),
        "wf2": rng.standard_normal((C3, 1), dtype=np.float32) * 0.1,
        "bf2": np.zeros(1, np.float32),
    }
    y = kernel(**demo)
    print("out", y.shape, y.dtype, y[:4, 0])
